# revision 17
# baseline (speedup 1.0000x reference)
"""Trainium2 Bass kernel for BottleneckAttention (patch attention).

q patches [160, 5120] from z1_hat (non-overlapping 10x4 unfold),
kv patches [5551, 5120] from z2 (overlapping unfold, Hk=91 x Wk=61),
scores = q @ kv.T / 5120, softmax over kv patches, out = attn @ kv,
folded back to [1, 128, 100, 64].

Sharding: contiguous blocks of 12 kv h-rows per core (8 x 12 = 96 >= 91).
Each core owns the 768 flat positions p = h_local*64 + w (w in [0,64);
positions with w >= 61 or h >= 91 are invalid -- their kv rows are zeroed
so they never touch the numerator, and the host subtracts their exactly
recomputed exp contribution from the denominator. Every core computes all
160 q rows; the host combines with an all-gather softmax.

Per-core kernel (raw Bass, explicit semaphores):
  phase 1 (bf16): scores as implicit convolution against the SBUF-resident
    z2 slice, streamed as CONTIGUOUS flat windows (the 3 junk columns per
    h-row avoid the strided-AP half-rate penalty on the PE).
  exp on ScalarE (scale = 1/5120), row-sum denominator on VectorE.
  PE transpose of exp-scores; the PSUM->SBUF copy applies bias=-1 so the
  bf16 e_T actually stores f = e-1 (centered softmax: |f| <~ 0.08 keeps
  absolute precision; the host adds the exact sum-of-kv-columns term).
  phase 2 (bf16): partial_out = f_T.T @ kv_shard, kv resident in SBUF.
"""

import sys

sys.path.insert(0, "/opt/trn_rl_repo")

import numpy as np
import ml_dtypes

import concourse.bass as bass
import concourse.mybir as mybir

DT = mybir.dt
AF = mybir.ActivationFunctionType

# problem geometry (hardcoded from the reference module)
KC, KH, KW = 128, 10, 4
H, W = 100, 64
NH, NW = H // KH, W // KW          # 10, 16
PQ = NH * NW                       # 160 q patches
D = KC * KH * KW                   # 5120
HK, WK = H - KH + 1, W - KW + 1    # 91, 61
NCORES = 8
HPC = 12                           # kv h-rows per core (8*12 = 96 >= 91)
PKC = HPC * W                      # 768 flat positions per core
T = 6                              # 768 / 128 k-chunks for phase 2
G0H, G1H = 7, 5                    # phase-1 h-groups (7+5 = 12)
N0 = G0H * W - (W - WK)            # 445 contiguous stream for h 0..6
N1 = G1H * W - (W - WK)            # 317 contiguous stream for h 7..11
OFF1 = G0H * W                     # 448: flat offset of group 1
ZROWS = 2 * HPC                    # 24 z rows staged per core
SCALE = 1.0 / D

P1_NP = ml_dtypes.bfloat16

_CACHE = {}


def _build_nc():
    nc = bass.Bass()
    z_d = nc.declare_dram_parameter("z", [KC, ZROWS * W], DT.bfloat16, isOutput=False)
    q_d = nc.declare_dram_parameter("qT3", [KC, KH * KW, PQ], DT.bfloat16, isOutput=False)
    kv_d = nc.declare_dram_parameter("kvr", [128, T, D], DT.bfloat16, isOutput=False)
    out_d = nc.declare_dram_parameter("out", [PQ, D], DT.float32, isOutput=True)
    den_d = nc.declare_dram_parameter("den", [PQ, 1], DT.float32, isOutput=True)

    from contextlib import ExitStack

    ctx = ExitStack()
    with ctx:
        z_sb = ctx.enter_context(nc.sbuf_tensor([KC, ZROWS * W], DT.bfloat16))
        q_sb = ctx.enter_context(nc.sbuf_tensor([KC, KH * KW, PQ], DT.bfloat16))
        kv_sb = ctx.enter_context(nc.sbuf_tensor([128, T, D], DT.bfloat16))
        e_hi = ctx.enter_context(nc.sbuf_tensor([128, PKC], DT.float32))
        e_lo = ctx.enter_context(nc.sbuf_tensor([32, PKC], DT.float32))
        eT_sb = ctx.enter_context(nc.sbuf_tensor([128, T, PQ], DT.bfloat16))
        o_hi = ctx.enter_context(nc.sbuf_tensor([128, D], DT.float32))
        o_lo = ctx.enter_context(nc.sbuf_tensor([32, D], DT.float32))
        iden = ctx.enter_context(nc.sbuf_tensor([128, 128], DT.float32))
        wz = ctx.enter_context(nc.sbuf_tensor([128, 512], DT.bfloat16))
        bias0 = ctx.enter_context(nc.sbuf_tensor([128, 1], DT.float32))
        dh_sb = ctx.enter_context(nc.sbuf_tensor([128, 1], DT.float32))
        dl_sb = ctx.enter_context(nc.sbuf_tensor([32, 1], DT.float32))

        # phase-1 score accumulators: (h-group, q-half)
        ps_s = [
            ctx.enter_context(nc.psum_tensor(f"ps_s{i}", [128, n], DT.float32))
            for i, n in enumerate((N0, N0, N1, N1))
        ]  # order: g0m0, g0m1, g1m0, g1m1
        # transpose staging / phase-2 accumulators (4 distinct banks)
        ps_t = [
            ctx.enter_context(nc.psum_tensor(f"ps_t{i}", [128, 512], DT.float32))
            for i in range(4)
        ]

        s_z = ctx.enter_context(nc.semaphore("s_z"))
        s_qq = [ctx.enter_context(nc.semaphore(f"s_qq{i}")) for i in range(4)]
        s_kv = ctx.enter_context(nc.semaphore("s_kv"))
        s_p = ctx.enter_context(nc.semaphore("s_p"))
        s_a = ctx.enter_context(nc.semaphore("s_a"))
        s_v = ctx.enter_context(nc.semaphore("s_v"))
        s_g = ctx.enter_context(nc.semaphore("s_g"))
        s_o = ctx.enter_context(nc.semaphore("s_o"))

        # 20 phase-2 output groups (m-half x n-tile of 512)
        NT = D // 512  # 10
        groups = [(m, n) for m in range(2) for n in range(NT)]
        # transpose order: finer-grained waits on the 4 exp calls
        # exp order: g0m0 (s_a=1), g0m1 (2), g1m0 (3), g1m1 (4)
        # chunk t<=2 only needs group 0; t>=3 spans group 1 too.
        tr_list = (
            [(t, 0, 1) for t in range(3)]
            + [(t, 1, 2) for t in range(3)]
            + [(t, 0, 3) for t in range(3, 6)]
            + [(t, 1, 4) for t in range(3, 6)]
        )  # (chunk, m, s_a threshold)

        with nc.Block() as block:

            @block.gpsimd
            def _(g):
                g.memset(wz[:], 0.0).then_inc(s_g, 1)        # 1: warmup tile
                g.memset(iden[:], 0.0)
                g.affine_select(
                    out=iden[:],
                    in_=iden[:],
                    compare_op=mybir.AluOpType.not_equal,
                    fill=1.0,
                    base=0,
                    pattern=[[-1, 128]],
                    channel_multiplier=1,
                ).then_inc(s_g, 1)                            # 2: identity
                g.memset(eT_sb[:], 0.0).then_inc(s_g, 1)      # 3: zero f_T
                # only the 6 uncovered columns per half need zeroing
                g.memset(e_hi[:, N0:OFF1], 0.0)
                g.memset(e_hi[:, OFF1 + N1 : PKC], 0.0)
                g.memset(e_lo[:, N0:OFF1], 0.0)
                g.memset(e_lo[:, OFF1 + N1 : PKC], 0.0)
                g.memset(bias0[:], 0.0).then_inc(s_g, 1)      # 4: e + bias

            @block.sync
            def _(sync):
                sync.dma_start(z_sb[:], z_d[:]).then_inc(s_z, 16)
                # q in quarters, each with its own semaphore (completion
                # order across DMA queues is not guaranteed)
                for qtr in range(4):
                    sl = slice(10 * qtr, 10 * qtr + 10)
                    sync.dma_start(q_sb[:, sl, :], q_d[:, sl, :]).then_inc(
                        s_qq[qtr], 16
                    )
                for c in range(3):
                    sync.dma_start(
                        kv_sb[:, 2 * c : 2 * c + 2, :], kv_d[:, 2 * c : 2 * c + 2, :]
                    ).then_inc(s_kv, 16)
                sync.wait_ge(s_v, 1)
                sync.dma_start(den_d[0:128, :], dh_sb[:]).then_inc(s_o, 16)
                sync.wait_ge(s_v, 2)
                sync.dma_start(den_d[128:160, :], dl_sb[:]).then_inc(s_o, 16)
                # out halves pipelined behind the ACT psum->sbuf copies
                # (out-copy g bumps s_a to 17+g; m0 tiles are g 0..9)
                sync.wait_ge(s_a, 21)
                sync.dma_start(out_d[0:128, 0:2560], o_hi[:, 0:2560]).then_inc(s_o, 16)
                sync.wait_ge(s_a, 26)
                sync.dma_start(out_d[0:128, 2560:], o_hi[:, 2560:]).then_inc(s_o, 16)
                sync.wait_ge(s_a, 31)
                sync.dma_start(out_d[128:160, 0:2560], o_lo[:, 0:2560]).then_inc(
                    s_o, 16
                )
                sync.wait_ge(s_a, 36)
                sync.dma_start(out_d[128:160, 2560:], o_lo[:, 2560:]).then_inc(s_o, 16)
                sync.wait_ge(s_o, 96)

            @block.tensor
            def _(pe):
                # HAM warmup on the zeroed bf16 tile while input DMAs land:
                # phase 1 then starts at the warm 2.4 GHz clock.
                pe.wait_ge(s_g, 1)
                for _w in range(14):
                    nc.tensor.matmul(
                        ps_t[0][0:128, 0:512],
                        wz[:, 0:128],
                        wz[:, 0:512],
                        start=True,
                        stop=True,
                    )
                pe.wait_ge(s_z, 16)
                pe.wait_ge(s_qq[0], 16)
                # phase 1: scores[pq, pos] += q(:,ij,:).T @ zflat[:, off+pos]
                # contiguous streams; junk cols (w>=61) corrected on host
                for ij in range(KH * KW):
                    if ij in (10, 20, 30):
                        pe.wait_ge(s_qq[ij // 10], 16)
                    i_, j_ = ij // KW, ij % KW
                    off = i_ * W + j_
                    st, sp = ij == 0, ij == KH * KW - 1
                    rhs0 = z_sb[:, off : off + N0]
                    rhs1 = z_sb[:, off + OFF1 : off + OFF1 + N1]
                    mm = nc.tensor.matmul(
                        ps_s[0][:, :], q_sb[:, ij, 0:128], rhs0, start=st, stop=sp
                    )
                    nc.tensor.matmul(
                        ps_s[1][0:32, :], q_sb[:, ij, 128:160], rhs0, start=st, stop=sp
                    )
                    nc.tensor.matmul(
                        ps_s[2][:, :], q_sb[:, ij, 0:128], rhs1, start=st, stop=sp
                    )
                    mm = nc.tensor.matmul(
                        ps_s[3][0:32, :], q_sb[:, ij, 128:160], rhs1, start=st, stop=sp
                    )
                mm.then_inc(s_p, 1)  # s_p = 1

                # transposes of exp-scores chunks -> ps_t (fp32)
                pe.wait_ge(s_g, 2)
                for k, (t, m, thr) in enumerate(tr_list):
                    msz = 128 if m == 0 else 32
                    src = (
                        e_hi[:, t * 128 : (t + 1) * 128]
                        if m == 0
                        else e_lo[:, t * 128 : (t + 1) * 128]
                    )
                    pe.wait_ge(s_a, thr if k < 4 else max(thr, k + 1))
                    nc.tensor.matmul(
                        ps_t[k % 4][0:128, 0:msz],
                        src,
                        iden[0:msz, 0:msz],
                        is_transpose=True,
                        start=True,
                        stop=True,
                    ).then_inc(s_p, 1)  # s_p = 2+k
                # phase 2: out[pq, d] = sum_t fT[., t, pq].T @ kv[., t, d]
                pe.wait_ge(s_g, 3)
                pe.wait_ge(s_a, 16)
                pe.wait_ge(s_kv, 48)
                for gidx, (m, n) in enumerate(groups):
                    m0, msz = (0, 128) if m == 0 else (128, 32)
                    if gidx >= 4:
                        pe.wait_ge(s_a, 13 + gidx)  # out-copy gidx-4 done
                    for t in range(T):
                        mm = nc.tensor.matmul(
                            ps_t[gidx % 4][0:msz, 0:512],
                            eT_sb[:, t, m0 : m0 + msz],
                            kv_sb[:, t, n * 512 : (n + 1) * 512],
                            start=(t == 0),
                            stop=(t == T - 1),
                        )
                    mm.then_inc(s_p, 1)  # s_p = 14+gidx

            @block.scalar
            def _(act):
                act.wait_ge(s_g, 4)
                act.wait_ge(s_p, 1)
                # exp(scores * SCALE) -> e (uncovered cols stay memset 0)
                nc.scalar.activation(
                    e_hi[:, 0:N0], ps_s[0][:, :], AF.Exp, bias=bias0[:, :], scale=SCALE
                ).then_inc(s_a, 1)
                nc.scalar.activation(
                    e_lo[:, 0:N0],
                    ps_s[1][0:32, :],
                    AF.Exp,
                    bias=bias0[0:32, :],
                    scale=SCALE,
                ).then_inc(s_a, 1)
                nc.scalar.activation(
                    e_hi[:, OFF1 : OFF1 + N1],
                    ps_s[2][:, :],
                    AF.Exp,
                    bias=bias0[:, :],
                    scale=SCALE,
                ).then_inc(s_a, 1)
                nc.scalar.activation(
                    e_lo[:, OFF1 : OFF1 + N1],
                    ps_s[3][0:32, :],
                    AF.Exp,
                    bias=bias0[0:32, :],
                    scale=SCALE,
                ).then_inc(s_a, 1)  # s_a = 4
                # copy transposed chunks into f_T = e - 1 (cast to bf16)
                for k, (t, m, _thr) in enumerate(tr_list):
                    m0, msz = (0, 128) if m == 0 else (128, 32)
                    act.wait_ge(s_p, 2 + k)
                    nc.scalar.activation(
                        eT_sb[:, t, m0 : m0 + msz],
                        ps_t[k % 4][0:128, 0:msz],
                        AF.Copy,
                        bias=-1.0,
                    ).then_inc(s_a, 1)  # s_a = 5+k
                # copy phase-2 accumulators to out staging
                for gidx, (m, n) in enumerate(groups):
                    msz = 128 if m == 0 else 32
                    dst = (
                        o_hi[:, n * 512 : (n + 1) * 512]
                        if m == 0
                        else o_lo[:, n * 512 : (n + 1) * 512]
                    )
                    act.wait_ge(s_p, 14 + gidx)
                    nc.scalar.activation(
                        dst, ps_t[gidx % 4][0:msz, 0:512], AF.Copy
                    ).then_inc(s_a, 1)  # s_a = 17+gidx

            @block.vector
            def _(dve):
                dve.wait_ge(s_a, 4)
                nc.vector.reduce_sum(
                    dh_sb[:], e_hi[:, :], axis=mybir.AxisListType.X
                ).then_inc(s_v, 1)
                nc.vector.reduce_sum(
                    dl_sb[:], e_lo[:, :], axis=mybir.AxisListType.X
                ).then_inc(s_v, 1)

    return nc


def _host_prep(z1_hat, z2):
    z1 = np.asarray(z1_hat, dtype=np.float32)[0]  # [128, 100, 64]
    z2a = np.asarray(z2, dtype=np.float32)[0]

    # q patches [160, 5120] and lhsT layout qT3 [128, 40, 160]
    q = z1.reshape(KC, NH, KH, NW, KW).transpose(1, 3, 0, 2, 4).reshape(PQ, D)
    qT3 = np.ascontiguousarray(
        q.reshape(PQ, KC, KH * KW).transpose(1, 2, 0).astype(P1_NP)
    )

    # padded z2: rows 100..111 zero
    z_pad = np.zeros((KC, 112, W), dtype=np.float32)
    z_pad[:, :H] = z2a

    # sliding kv patches from padded z2
    sw = np.lib.stride_tricks.sliding_window_view(z_pad, (KH, KW), axis=(1, 2))
    # sw: [128, 103, 61, 10, 4]; patch(h, w) = sw[:, h, w]

    q64 = q.astype(np.float64)
    ij_off = (np.arange(KH)[:, None] * W + np.arange(KW)[None, :]).reshape(-1)  # [40]

    in_maps = []
    corrs = []
    for core in range(NCORES):
        h0 = HPC * core
        zf = z_pad[:, h0 : h0 + ZROWS, :].reshape(KC, ZROWS * W)
        # kv rows indexed by flat position p = h_local*64 + w
        kvp = np.zeros((PKC, D), dtype=np.float32)
        hh = np.arange(PKC) // W
        ww = np.arange(PKC) % W
        real = (ww < WK) & (h0 + hh < HK)
        ridx = np.nonzero(real)[0]
        kvp[ridx] = (
            sw[:, h0 + hh[ridx], ww[ridx]].transpose(1, 0, 2, 3).reshape(-1, D)
        )
        kvr = np.ascontiguousarray(
            kvp.reshape(T, 128, D).transpose(1, 0, 2).astype(ml_dtypes.bfloat16)
        )
        in_maps.append(
            {
                "z": np.ascontiguousarray(zf.astype(P1_NP)),
                "qT3": qT3,
                "kvr": kvr,
            }
        )
        # denominator correction: computed-but-invalid columns. The device
        # computes exp(q . window / D) for every position in the two
        # contiguous streams [0,445) and [448,765); positions that are not
        # real patches (w >= 61 or h >= 91) polluted the on-chip row-sum.
        covered = np.zeros(PKC, dtype=bool)
        covered[0:N0] = True
        covered[OFF1 : OFF1 + N1] = True
        bad = np.nonzero(covered & ~real)[0]
        win = zf.astype(np.float64)[:, bad[:, None] + ij_off[None, :]]  # [128,nb,40]
        patches = win.transpose(1, 0, 2).reshape(len(bad), D)  # d-order (c, i, j)
        s_bad = q64 @ patches.T  # [160, nb]
        corrs.append(np.exp(s_bad * SCALE).sum(axis=1))

    corr = np.sum(corrs, axis=0)
    # centered softmax: device returns f @ kv with f = e - 1; host adds the
    # exact colsum term sum_k kv[k, :] over all real patches (all cores).
    swr = sw[:, :HK, :WK]
    colsum = swr.astype(np.float64).sum(axis=(1, 2)).reshape(D)  # [5120]
    return in_maps, corr, colsum


def kernel(z1_hat, z2):
    from concourse.bass_utils import run_bass_kernel_spmd

    in_maps, corr, colsum = _host_prep(z1_hat, z2)
    if "nc" not in _CACHE:
        _CACHE["nc"] = _build_nc()
    nc = _CACHE["nc"]
    res = run_bass_kernel_spmd(nc, in_maps, list(range(NCORES)))
    num = np.broadcast_to(colsum, (PQ, D)).astype(np.float64).copy()
    den = -corr
    for r in res.results:
        num += r["out"].astype(np.float64)
        den = den + r["den"].astype(np.float64)[:, 0]
    out = (num / den[:, None]).astype(np.float32)
    # fold patches back: [160, 5120] -> [1, 128, 100, 64]
    out = out.reshape(NH, NW, KC, KH, KW).transpose(2, 0, 3, 1, 4)
    return np.ascontiguousarray(out.reshape(1, KC, H, W))


# revision 18
# speedup vs baseline: 1.1702x; 1.1702x over previous
"""Trainium2 Bass kernel for BottleneckAttention (patch attention).

q patches [160, 5120] from z1_hat (non-overlapping 10x4 unfold),
kv patches [5551, 5120] from z2 (overlapping unfold, Hk=91 x Wk=61),
scores = q @ kv.T / 5120, softmax over kv patches, out = attn @ kv,
folded back to [1, 128, 100, 64].

Sharding: contiguous blocks of 12 kv h-rows per core (8 x 12 = 96 >= 91).
Each core owns the 768 flat positions p = h_local*64 + w (w in [0,64);
positions with w >= 61 or h >= 91 are invalid -- their kv rows are zeroed
so they never touch the numerator, and the host subtracts their exactly
recomputed exp contribution from the denominator. Every core computes all
160 q rows; the host combines with an all-gather softmax.

Per-core kernel (raw Bass, explicit semaphores):
  phase 1 (bf16): scores as implicit convolution against the SBUF-resident
    z2 slice, streamed as CONTIGUOUS flat windows (the 3 junk columns per
    h-row avoid the strided-AP half-rate penalty on the PE).
  exp on ScalarE (scale = 1/5120), row-sum denominator on VectorE.
  PE transpose of exp-scores; the PSUM->SBUF copy applies bias=-1 so the
  bf16 e_T actually stores f = e-1 (centered softmax: |f| <~ 0.08 keeps
  absolute precision; the host adds the exact sum-of-kv-columns term).
  phase 2 (bf16): partial_out = f_T.T @ kv_shard, kv resident in SBUF.
"""

import sys

sys.path.insert(0, "/opt/trn_rl_repo")

import numpy as np
import ml_dtypes

import concourse.bass as bass
import concourse.mybir as mybir

DT = mybir.dt
AF = mybir.ActivationFunctionType

# problem geometry (hardcoded from the reference module)
KC, KH, KW = 128, 10, 4
H, W = 100, 64
NH, NW = H // KH, W // KW          # 10, 16
PQ = NH * NW                       # 160 q patches
D = KC * KH * KW                   # 5120
HK, WK = H - KH + 1, W - KW + 1    # 91, 61
NCORES = 8
HPC = 12                           # kv h-rows per core (8*12 = 96 >= 91)
PKC = HPC * W                      # 768 flat positions per core
T = 6                              # 768 / 128 k-chunks for phase 2
G0H, G1H = 7, 5                    # phase-1 h-groups (7+5 = 12)
N0 = G0H * W                       # 448: contiguous stream for h 0..6
N1 = G1H * W                       # 320: contiguous stream for h 7..11
OFF1 = G0H * W                     # 448: flat offset of group 1
ZROWS = 2 * HPC                    # 24 z rows staged per core
SCALE = 1.0 / D

P1_NP = ml_dtypes.bfloat16

_CACHE = {}


def _build_nc():
    nc = bass.Bass()
    z_d = nc.declare_dram_parameter("z", [KC, KW, ZROWS * W], DT.bfloat16, isOutput=False)
    q_d = nc.declare_dram_parameter("qT3", [KC, KH * KW, PQ], DT.bfloat16, isOutput=False)
    kv_d = nc.declare_dram_parameter("kvr", [128, T, D], DT.bfloat16, isOutput=False)
    out_d = nc.declare_dram_parameter("out", [PQ, D], DT.float32, isOutput=True)
    den_d = nc.declare_dram_parameter("den", [PQ, 1], DT.float32, isOutput=True)

    from contextlib import ExitStack

    ctx = ExitStack()
    with ctx:
        # 4 byte-shifted copies of flat z so every (i,j) stream is 128B-aligned
        z_sb = ctx.enter_context(nc.sbuf_tensor([KC, KW, ZROWS * W], DT.bfloat16))
        q_sb = ctx.enter_context(nc.sbuf_tensor([KC, KH * KW, PQ], DT.bfloat16))
        kv_sb = ctx.enter_context(nc.sbuf_tensor([128, T, D], DT.bfloat16))
        e_hi = ctx.enter_context(nc.sbuf_tensor([128, PKC], DT.float32))
        e_lo = ctx.enter_context(nc.sbuf_tensor([32, PKC], DT.float32))
        eT_sb = ctx.enter_context(nc.sbuf_tensor([128, T, PQ], DT.bfloat16))
        o_hi = ctx.enter_context(nc.sbuf_tensor([128, D], DT.float32))
        o_lo = ctx.enter_context(nc.sbuf_tensor([32, D], DT.float32))
        iden = ctx.enter_context(nc.sbuf_tensor([128, 128], DT.float32))
        wz = ctx.enter_context(nc.sbuf_tensor([128, 512], DT.bfloat16))
        bias0 = ctx.enter_context(nc.sbuf_tensor([128, 1], DT.float32))
        dh_sb = ctx.enter_context(nc.sbuf_tensor([128, 1], DT.float32))
        dl_sb = ctx.enter_context(nc.sbuf_tensor([32, 1], DT.float32))

        # phase-1 score accumulators: (h-group, q-half)
        ps_s = [
            ctx.enter_context(nc.psum_tensor(f"ps_s{i}", [128, n], DT.float32))
            for i, n in enumerate((N0, N0, N1, N1))
        ]  # order: g0m0, g0m1, g1m0, g1m1
        # transpose staging / phase-2 accumulators (4 distinct banks)
        ps_t = [
            ctx.enter_context(nc.psum_tensor(f"ps_t{i}", [128, 512], DT.float32))
            for i in range(4)
        ]

        s_z = ctx.enter_context(nc.semaphore("s_z"))
        s_qq = [ctx.enter_context(nc.semaphore(f"s_qq{i}")) for i in range(4)]
        s_kv = ctx.enter_context(nc.semaphore("s_kv"))
        s_p = ctx.enter_context(nc.semaphore("s_p"))
        s_a = ctx.enter_context(nc.semaphore("s_a"))
        s_v = ctx.enter_context(nc.semaphore("s_v"))
        s_g = ctx.enter_context(nc.semaphore("s_g"))
        s_o = ctx.enter_context(nc.semaphore("s_o"))

        # 20 phase-2 output groups (m-half x n-tile of 512)
        NT = D // 512  # 10
        groups = [(m, n) for m in range(2) for n in range(NT)]
        # transpose order: finer-grained waits on the 4 exp calls
        # exp order: g0m0 (s_a=1), g0m1 (2), g1m0 (3), g1m1 (4)
        # chunk t<=2 only needs group 0; t>=3 spans group 1 too.
        tr_list = (
            [(t, 0, 1) for t in range(3)]
            + [(t, 1, 2) for t in range(3)]
            + [(t, 0, 3) for t in range(3, 6)]
            + [(t, 1, 4) for t in range(3, 6)]
        )  # (chunk, m, s_a threshold)

        with nc.Block() as block:

            @block.gpsimd
            def _(g):
                g.memset(wz[:], 0.0).then_inc(s_g, 1)        # 1: warmup tile
                g.memset(iden[:], 0.0)
                g.affine_select(
                    out=iden[:],
                    in_=iden[:],
                    compare_op=mybir.AluOpType.not_equal,
                    fill=1.0,
                    base=0,
                    pattern=[[-1, 128]],
                    channel_multiplier=1,
                ).then_inc(s_g, 1)                            # 2: identity
                g.memset(bias0[:], 0.0).then_inc(s_g, 1)      # 3: bias

            @block.sync
            def _(sync):
                sync.dma_start(z_sb[:], z_d[:]).then_inc(s_z, 16)
                # q in quarters, each with its own semaphore (completion
                # order across DMA queues is not guaranteed)
                for qtr in range(4):
                    sl = slice(10 * qtr, 10 * qtr + 10)
                    sync.dma_start(q_sb[:, sl, :], q_d[:, sl, :]).then_inc(
                        s_qq[qtr], 16
                    )
                for c in range(3):
                    sync.dma_start(
                        kv_sb[:, 2 * c : 2 * c + 2, :], kv_d[:, 2 * c : 2 * c + 2, :]
                    ).then_inc(s_kv, 16)
                sync.wait_ge(s_v, 1)
                sync.dma_start(den_d[0:128, :], dh_sb[:]).then_inc(s_o, 16)
                sync.wait_ge(s_v, 2)
                sync.dma_start(den_d[128:160, :], dl_sb[:]).then_inc(s_o, 16)
                # out halves pipelined behind the ACT psum->sbuf copies
                # (out-copy g bumps s_a to 17+g; m0 tiles are g 0..9)
                sync.wait_ge(s_a, 21)
                sync.dma_start(out_d[0:128, 0:2560], o_hi[:, 0:2560]).then_inc(s_o, 16)
                sync.wait_ge(s_a, 26)
                sync.dma_start(out_d[0:128, 2560:], o_hi[:, 2560:]).then_inc(s_o, 16)
                sync.wait_ge(s_a, 31)
                sync.dma_start(out_d[128:160, 0:2560], o_lo[:, 0:2560]).then_inc(
                    s_o, 16
                )
                sync.wait_ge(s_a, 36)
                sync.dma_start(out_d[128:160, 2560:], o_lo[:, 2560:]).then_inc(s_o, 16)
                sync.wait_ge(s_o, 96)

            @block.tensor
            def _(pe):
                # HAM warmup on the zeroed bf16 tile while input DMAs land:
                # phase 1 then starts at the warm 2.4 GHz clock.
                pe.wait_ge(s_g, 1)
                for _w in range(14):
                    nc.tensor.matmul(
                        ps_t[0][0:128, 0:512],
                        wz[:, 0:128],
                        wz[:, 0:512],
                        start=True,
                        stop=True,
                    )
                pe.wait_ge(s_z, 16)
                pe.wait_ge(s_qq[0], 16)
                # phase 1: scores[pq, pos] += q(:,ij,:).T @ zflat[:, off+pos]
                # contiguous streams; junk cols (w>=61) corrected on host
                for ij in range(KH * KW):
                    if ij in (10, 20, 30):
                        pe.wait_ge(s_qq[ij // 10], 16)
                    i_, j_ = ij // KW, ij % KW
                    st, sp = ij == 0, ij == KH * KW - 1
                    rhs0 = z_sb[:, j_, i_ * W : i_ * W + N0]
                    rhs1 = z_sb[:, j_, i_ * W + OFF1 : i_ * W + OFF1 + N1]
                    mm = nc.tensor.matmul(
                        ps_s[0][:, :], q_sb[:, ij, 0:128], rhs0, start=st, stop=sp
                    )
                    nc.tensor.matmul(
                        ps_s[1][0:32, :], q_sb[:, ij, 128:160], rhs0, start=st, stop=sp
                    )
                    nc.tensor.matmul(
                        ps_s[2][:, :], q_sb[:, ij, 0:128], rhs1, start=st, stop=sp
                    )
                    mm = nc.tensor.matmul(
                        ps_s[3][0:32, :], q_sb[:, ij, 128:160], rhs1, start=st, stop=sp
                    )
                mm.then_inc(s_p, 1)  # s_p = 1

                # transposes of exp-scores chunks -> ps_t (fp32)
                pe.wait_ge(s_g, 2)
                for k, (t, m, thr) in enumerate(tr_list):
                    msz = 128 if m == 0 else 32
                    src = (
                        e_hi[:, t * 128 : (t + 1) * 128]
                        if m == 0
                        else e_lo[:, t * 128 : (t + 1) * 128]
                    )
                    pe.wait_ge(s_a, thr if k < 4 else max(thr, k + 1))
                    nc.tensor.matmul(
                        ps_t[k % 4][0:128, 0:msz],
                        src,
                        iden[0:msz, 0:msz],
                        is_transpose=True,
                        start=True,
                        stop=True,
                    ).then_inc(s_p, 1)  # s_p = 2+k
                # phase 2: out[pq, d] = sum_t fT[., t, pq].T @ kv[., t, d]
                pe.wait_ge(s_a, 16)
                pe.wait_ge(s_kv, 48)
                for gidx, (m, n) in enumerate(groups):
                    m0, msz = (0, 128) if m == 0 else (128, 32)
                    if gidx >= 4:
                        pe.wait_ge(s_a, 13 + gidx)  # out-copy gidx-4 done
                    for t in range(T):
                        mm = nc.tensor.matmul(
                            ps_t[gidx % 4][0:msz, 0:512],
                            eT_sb[:, t, m0 : m0 + msz],
                            kv_sb[:, t, n * 512 : (n + 1) * 512],
                            start=(t == 0),
                            stop=(t == T - 1),
                        )
                    mm.then_inc(s_p, 1)  # s_p = 14+gidx

            @block.scalar
            def _(act):
                act.wait_ge(s_g, 3)
                act.wait_ge(s_p, 1)
                # exp(scores * SCALE) -> e (uncovered cols stay memset 0)
                nc.scalar.activation(
                    e_hi[:, 0:N0], ps_s[0][:, :], AF.Exp, bias=bias0[:, :], scale=SCALE
                ).then_inc(s_a, 1)
                nc.scalar.activation(
                    e_lo[:, 0:N0],
                    ps_s[1][0:32, :],
                    AF.Exp,
                    bias=bias0[0:32, :],
                    scale=SCALE,
                ).then_inc(s_a, 1)
                nc.scalar.activation(
                    e_hi[:, OFF1 : OFF1 + N1],
                    ps_s[2][:, :],
                    AF.Exp,
                    bias=bias0[:, :],
                    scale=SCALE,
                ).then_inc(s_a, 1)
                nc.scalar.activation(
                    e_lo[:, OFF1 : OFF1 + N1],
                    ps_s[3][0:32, :],
                    AF.Exp,
                    bias=bias0[0:32, :],
                    scale=SCALE,
                ).then_inc(s_a, 1)  # s_a = 4
                # copy transposed chunks into f_T = e - 1 (cast to bf16)
                for k, (t, m, _thr) in enumerate(tr_list):
                    m0, msz = (0, 128) if m == 0 else (128, 32)
                    act.wait_ge(s_p, 2 + k)
                    nc.scalar.activation(
                        eT_sb[:, t, m0 : m0 + msz],
                        ps_t[k % 4][0:128, 0:msz],
                        AF.Copy,
                        bias=-1.0,
                    ).then_inc(s_a, 1)  # s_a = 5+k
                # copy phase-2 accumulators to out staging
                for gidx, (m, n) in enumerate(groups):
                    msz = 128 if m == 0 else 32
                    dst = (
                        o_hi[:, n * 512 : (n + 1) * 512]
                        if m == 0
                        else o_lo[:, n * 512 : (n + 1) * 512]
                    )
                    act.wait_ge(s_p, 14 + gidx)
                    nc.scalar.activation(
                        dst, ps_t[gidx % 4][0:msz, 0:512], AF.Copy
                    ).then_inc(s_a, 1)  # s_a = 17+gidx

            @block.vector
            def _(dve):
                dve.wait_ge(s_a, 4)
                nc.vector.reduce_sum(
                    dh_sb[:], e_hi[:, :], axis=mybir.AxisListType.X
                ).then_inc(s_v, 1)
                nc.vector.reduce_sum(
                    dl_sb[:], e_lo[:, :], axis=mybir.AxisListType.X
                ).then_inc(s_v, 1)

    return nc


def _host_prep(z1_hat, z2):
    z1 = np.asarray(z1_hat, dtype=np.float32)[0]  # [128, 100, 64]
    z2a = np.asarray(z2, dtype=np.float32)[0]

    # q patches [160, 5120] and lhsT layout qT3 [128, 40, 160]
    q = z1.reshape(KC, NH, KH, NW, KW).transpose(1, 3, 0, 2, 4).reshape(PQ, D)
    qT3 = np.ascontiguousarray(
        q.reshape(PQ, KC, KH * KW).transpose(1, 2, 0).astype(P1_NP)
    )

    # padded z2: rows 100..111 zero
    z_pad = np.zeros((KC, 112, W), dtype=np.float32)
    z_pad[:, :H] = z2a

    # sliding kv patches from padded z2
    sw = np.lib.stride_tricks.sliding_window_view(z_pad, (KH, KW), axis=(1, 2))
    # sw: [128, 103, 61, 10, 4]; patch(h, w) = sw[:, h, w]

    q64 = q.astype(np.float64)
    ij_off = (np.arange(KH)[:, None] * W + np.arange(KW)[None, :]).reshape(-1)  # [40]

    in_maps = []
    corrs = []
    for core in range(NCORES):
        h0 = HPC * core
        zf = z_pad[:, h0 : h0 + ZROWS, :].reshape(KC, ZROWS * W)
        # 4 byte-shifted slabs for 128B-aligned phase-1 streams
        z4 = np.zeros((KC, KW, ZROWS * W), dtype=np.float32)
        for s in range(KW):
            z4[:, s, : ZROWS * W - s] = zf[:, s:]
        # kv rows indexed by flat position p = h_local*64 + w
        kvp = np.zeros((PKC, D), dtype=np.float32)
        hh = np.arange(PKC) // W
        ww = np.arange(PKC) % W
        real = (ww < WK) & (h0 + hh < HK)
        ridx = np.nonzero(real)[0]
        kvp[ridx] = (
            sw[:, h0 + hh[ridx], ww[ridx]].transpose(1, 0, 2, 3).reshape(-1, D)
        )
        kvr = np.ascontiguousarray(
            kvp.reshape(T, 128, D).transpose(1, 0, 2).astype(ml_dtypes.bfloat16)
        )
        in_maps.append(
            {
                "z": np.ascontiguousarray(z4.astype(P1_NP)),
                "qT3": qT3,
                "kvr": kvr,
            }
        )
        # denominator correction: computed-but-invalid columns. The device
        # computes exp(q . window / D) for every position in the two
        # contiguous streams [0,445) and [448,765); positions that are not
        # real patches (w >= 61 or h >= 91) polluted the on-chip row-sum.
        # streams now cover every flat position; invalid = not a real patch
        bad = np.nonzero(~real)[0]
        win = zf.astype(np.float64)[:, bad[:, None] + ij_off[None, :]]  # [128,nb,40]
        patches = win.transpose(1, 0, 2).reshape(len(bad), D)  # d-order (c, i, j)
        s_bad = q64 @ patches.T  # [160, nb]
        corrs.append(np.exp(s_bad * SCALE).sum(axis=1))

    corr = np.sum(corrs, axis=0)
    # centered softmax: device returns f @ kv with f = e - 1; host adds the
    # exact colsum term sum_k kv[k, :] over all real patches (all cores).
    swr = sw[:, :HK, :WK]
    colsum = swr.astype(np.float64).sum(axis=(1, 2)).reshape(D)  # [5120]
    return in_maps, corr, colsum


def kernel(z1_hat, z2):
    from concourse.bass_utils import run_bass_kernel_spmd

    in_maps, corr, colsum = _host_prep(z1_hat, z2)
    if "nc" not in _CACHE:
        _CACHE["nc"] = _build_nc()
    nc = _CACHE["nc"]
    res = run_bass_kernel_spmd(nc, in_maps, list(range(NCORES)))
    num = np.broadcast_to(colsum, (PQ, D)).astype(np.float64).copy()
    den = -corr
    for r in res.results:
        num += r["out"].astype(np.float64)
        den = den + r["den"].astype(np.float64)[:, 0]
    out = (num / den[:, None]).astype(np.float32)
    # fold patches back: [160, 5120] -> [1, 128, 100, 64]
    out = out.reshape(NH, NW, KC, KH, KW).transpose(2, 0, 3, 1, 4)
    return np.ascontiguousarray(out.reshape(1, KC, H, W))


# revision 19
# speedup vs baseline: 1.5043x; 1.2855x over previous
"""Trainium2 Bass kernel for BottleneckAttention (patch attention).

q patches [160, 5120] from z1_hat (non-overlapping 10x4 unfold),
kv patches [5551, 5120] from z2 (overlapping unfold, Hk=91 x Wk=61),
scores = q @ kv.T / 5120, softmax over kv patches, out = attn @ kv,
folded back to [1, 128, 100, 64].

Sharding: contiguous blocks of 12 kv h-rows per core (8 x 12 = 96 >= 91).
Each core owns the 768 flat positions p = h_local*64 + w (w in [0,64);
positions with w >= 61 or h >= 91 are invalid -- their kv rows are zeroed
so they never touch the numerator, and the host subtracts their exactly
recomputed exp contribution from the denominator. Every core computes all
160 q rows; the host combines with an all-gather softmax.

Per-core kernel (raw Bass, explicit semaphores):
  phase 1 (bf16): scores as implicit convolution against the SBUF-resident
    z2 slice, streamed as CONTIGUOUS flat windows (the 3 junk columns per
    h-row avoid the strided-AP half-rate penalty on the PE).
  exp on ScalarE (scale = 1/5120), row-sum denominator on VectorE.
  PE transpose of exp-scores; the PSUM->SBUF copy applies bias=-1 so the
  bf16 e_T actually stores f = e-1 (centered softmax: |f| <~ 0.08 keeps
  absolute precision; the host adds the exact sum-of-kv-columns term).
  phase 2 (bf16): partial_out = f_T.T @ kv_shard, kv resident in SBUF.
"""

import sys

sys.path.insert(0, "/opt/trn_rl_repo")

import numpy as np
import ml_dtypes

import concourse.bass as bass
import concourse.mybir as mybir

DT = mybir.dt
AF = mybir.ActivationFunctionType

# problem geometry (hardcoded from the reference module)
KC, KH, KW = 128, 10, 4
H, W = 100, 64
NH, NW = H // KH, W // KW          # 10, 16
PQ = NH * NW                       # 160 q patches
D = KC * KH * KW                   # 5120
HK, WK = H - KH + 1, W - KW + 1    # 91, 61
NCORES = 8
HPC = 12                           # kv h-rows per core (8*12 = 96 >= 91)
PKC = HPC * W                      # 768 flat positions per core
T = 6                              # 768 / 128 k-chunks for phase 2
G0H, G1H = 7, 5                    # phase-1 h-groups (7+5 = 12)
N0 = G0H * W                       # 448: contiguous stream for h 0..6
N1 = G1H * W                       # 320: contiguous stream for h 7..11
OFF1 = G0H * W                     # 448: flat offset of group 1
ZROWS = 2 * HPC                    # 24 z rows staged per core
SCALE = 1.0 / D

P1_NP = ml_dtypes.bfloat16

_CACHE = {}


def _build_nc():
    nc = bass.Bass()
    z_d = nc.declare_dram_parameter("z", [KC, KW, ZROWS * W], DT.bfloat16, isOutput=False)
    q_d = nc.declare_dram_parameter("qT3", [KC, KH * KW, PQ], DT.bfloat16, isOutput=False)
    kv_d = nc.declare_dram_parameter("kvr", [128, T, D], DT.bfloat16, isOutput=False)
    out_d = nc.declare_dram_parameter("out", [PQ, D], DT.float32, isOutput=True)
    den_d = nc.declare_dram_parameter("den", [PQ, 1], DT.float32, isOutput=True)

    from contextlib import ExitStack

    ctx = ExitStack()
    with ctx:
        # 4 byte-shifted copies of flat z so every (i,j) stream is 128B-aligned
        z_sb = ctx.enter_context(nc.sbuf_tensor([KC, KW, ZROWS * W], DT.bfloat16))
        q_sb = ctx.enter_context(nc.sbuf_tensor([KC, KH * KW, PQ], DT.bfloat16))
        kv_sb = ctx.enter_context(nc.sbuf_tensor([128, T, D], DT.bfloat16))
        e_hi = ctx.enter_context(nc.sbuf_tensor([128, PKC], DT.float32))
        e_lo = ctx.enter_context(nc.sbuf_tensor([32, PKC], DT.float32))
        eT_sb = ctx.enter_context(nc.sbuf_tensor([128, T, PQ], DT.bfloat16))
        o_hi = ctx.enter_context(nc.sbuf_tensor([128, D], DT.float32))
        o_lo = ctx.enter_context(nc.sbuf_tensor([32, D], DT.float32))
        iden = ctx.enter_context(nc.sbuf_tensor([128, 128], DT.float32))
        wz = ctx.enter_context(nc.sbuf_tensor([128, 512], DT.bfloat16))
        bias0 = ctx.enter_context(nc.sbuf_tensor([128, 1], DT.float32))
        dh_sb = ctx.enter_context(nc.sbuf_tensor([128, 1], DT.float32))
        dl_sb = ctx.enter_context(nc.sbuf_tensor([32, 1], DT.float32))

        # phase-1 score accumulators: (h-group, q-half)
        ps_s = [
            ctx.enter_context(nc.psum_tensor(f"ps_s{i}", [128, n], DT.float32))
            for i, n in enumerate((N0, N0, N1, N1))
        ]  # order: g0m0, g0m1, g1m0, g1m1
        # transpose staging / phase-2 accumulators (4 distinct banks)
        ps_t = [
            ctx.enter_context(nc.psum_tensor(f"ps_t{i}", [128, 512], DT.float32))
            for i in range(4)
        ]

        s_z = ctx.enter_context(nc.semaphore("s_z"))
        s_qq = [ctx.enter_context(nc.semaphore(f"s_qq{i}")) for i in range(4)]
        s_kv = ctx.enter_context(nc.semaphore("s_kv"))
        s_p = ctx.enter_context(nc.semaphore("s_p"))
        s_a = ctx.enter_context(nc.semaphore("s_a"))
        s_v = ctx.enter_context(nc.semaphore("s_v"))
        s_g = ctx.enter_context(nc.semaphore("s_g"))
        s_o = ctx.enter_context(nc.semaphore("s_o"))

        # 20 phase-2 output groups (m-half x n-tile of 512)
        NT = D // 512  # 10
        groups = [(m, n) for m in range(2) for n in range(NT)]
        # transpose order: finer-grained waits on the 4 exp calls
        # exp order: g0m0 (s_a=1), g0m1 (2), g1m0 (3), g1m1 (4)
        # chunk t<=2 only needs group 0; t>=3 spans group 1 too.
        tr_list = (
            [(t, 0, 1) for t in range(3)]
            + [(t, 1, 2) for t in range(3)]
            + [(t, 0, 3) for t in range(3, 6)]
            + [(t, 1, 4) for t in range(3, 6)]
        )  # (chunk, m, s_a threshold)

        with nc.Block() as block:

            @block.gpsimd
            def _(g):
                g.memset(wz[:], 0.0).then_inc(s_g, 1)        # 1: warmup tile
                g.memset(iden[:], 0.0)
                g.affine_select(
                    out=iden[:],
                    in_=iden[:],
                    compare_op=mybir.AluOpType.not_equal,
                    fill=1.0,
                    base=0,
                    pattern=[[-1, 128]],
                    channel_multiplier=1,
                ).then_inc(s_g, 1)                            # 2: identity
                g.memset(bias0[:], 0.0).then_inc(s_g, 1)      # 3: bias

            @block.sync
            def _(sync):
                sync.dma_start(z_sb[:], z_d[:]).then_inc(s_z, 16)
                # q in quarters, each with its own semaphore (completion
                # order across DMA queues is not guaranteed)
                for qtr in range(4):
                    sl = slice(10 * qtr, 10 * qtr + 10)
                    sync.dma_start(q_sb[:, sl, :], q_d[:, sl, :]).then_inc(
                        s_qq[qtr], 16
                    )
                for c in range(3):
                    sync.dma_start(
                        kv_sb[:, 2 * c : 2 * c + 2, :], kv_d[:, 2 * c : 2 * c + 2, :]
                    ).then_inc(s_kv, 16)
                sync.wait_ge(s_v, 1)
                sync.dma_start(den_d[0:128, :], dh_sb[:]).then_inc(s_o, 16)
                sync.wait_ge(s_v, 2)
                sync.dma_start(den_d[128:160, :], dl_sb[:]).then_inc(s_o, 16)
                # out halves pipelined behind the ACT psum->sbuf copies
                # (out-copy g bumps s_a to 17+g; m0 tiles are g 0..9)
                sync.wait_ge(s_a, 21)
                sync.dma_start(out_d[0:128, 0:2560], o_hi[:, 0:2560]).then_inc(s_o, 16)
                sync.wait_ge(s_a, 26)
                sync.dma_start(out_d[0:128, 2560:], o_hi[:, 2560:]).then_inc(s_o, 16)
                sync.wait_ge(s_a, 31)
                sync.dma_start(out_d[128:160, 0:2560], o_lo[:, 0:2560]).then_inc(
                    s_o, 16
                )
                sync.wait_ge(s_a, 36)
                sync.dma_start(out_d[128:160, 2560:], o_lo[:, 2560:]).then_inc(s_o, 16)
                sync.wait_ge(s_o, 96)

            @block.tensor
            def _(pe):
                # HAM warmup on the zeroed bf16 tile while input DMAs land:
                # phase 1 then starts at the warm 2.4 GHz clock.
                pe.wait_ge(s_g, 1)
                for _w in range(14):
                    nc.tensor.matmul(
                        ps_t[0][0:128, 0:512],
                        wz[:, 0:128],
                        wz[:, 0:512],
                        start=True,
                        stop=True,
                    )
                pe.wait_ge(s_z, 16)
                pe.wait_ge(s_qq[0], 16)
                # phase 1: scores[pq, pos] += q(:,ij,:).T @ zflat[:, off+pos]
                # contiguous streams; junk cols (w>=61) corrected on host.
                # One long accumulation chain per psum group -- the PE only
                # pipelines back-to-back matmuls within a group, so group-
                # rotating per ij runs ~2x slower.
                for gi, (grp, m) in enumerate([(0, 0), (0, 1), (1, 0), (1, 1)]):
                    ps = ps_s[grp * 2 + m]
                    dst = ps[:, :] if m == 0 else ps[0:32, :]
                    msl = slice(0, 128) if m == 0 else slice(128, 160)
                    for ij in range(KH * KW):
                        if gi == 0 and ij in (10, 20, 30):
                            pe.wait_ge(s_qq[ij // 10], 16)
                        i_, j_ = ij // KW, ij % KW
                        st, sp = ij == 0, ij == KH * KW - 1
                        base = i_ * W + (OFF1 if grp == 1 else 0)
                        rhs = z_sb[:, j_, base : base + (N1 if grp == 1 else N0)]
                        mm = nc.tensor.matmul(
                            dst, q_sb[:, ij, msl], rhs, start=st, stop=sp
                        )
                mm.then_inc(s_p, 1)  # s_p = 1

                # transposes of exp-scores chunks -> ps_t (fp32)
                pe.wait_ge(s_g, 2)
                for k, (t, m, thr) in enumerate(tr_list):
                    msz = 128 if m == 0 else 32
                    src = (
                        e_hi[:, t * 128 : (t + 1) * 128]
                        if m == 0
                        else e_lo[:, t * 128 : (t + 1) * 128]
                    )
                    pe.wait_ge(s_a, thr if k < 4 else max(thr, k + 1))
                    nc.tensor.matmul(
                        ps_t[k % 4][0:128, 0:msz],
                        src,
                        iden[0:msz, 0:msz],
                        is_transpose=True,
                        start=True,
                        stop=True,
                    ).then_inc(s_p, 1)  # s_p = 2+k
                # phase 2: out[pq, d] = sum_t fT[., t, pq].T @ kv[., t, d]
                pe.wait_ge(s_a, 16)
                pe.wait_ge(s_kv, 48)
                for gidx, (m, n) in enumerate(groups):
                    m0, msz = (0, 128) if m == 0 else (128, 32)
                    if gidx >= 4:
                        pe.wait_ge(s_a, 13 + gidx)  # out-copy gidx-4 done
                    for t in range(T):
                        mm = nc.tensor.matmul(
                            ps_t[gidx % 4][0:msz, 0:512],
                            eT_sb[:, t, m0 : m0 + msz],
                            kv_sb[:, t, n * 512 : (n + 1) * 512],
                            start=(t == 0),
                            stop=(t == T - 1),
                        )
                    mm.then_inc(s_p, 1)  # s_p = 14+gidx

            @block.scalar
            def _(act):
                act.wait_ge(s_g, 3)
                act.wait_ge(s_p, 1)
                # exp(scores * SCALE) -> e (uncovered cols stay memset 0)
                nc.scalar.activation(
                    e_hi[:, 0:N0], ps_s[0][:, :], AF.Exp, bias=bias0[:, :], scale=SCALE
                ).then_inc(s_a, 1)
                nc.scalar.activation(
                    e_lo[:, 0:N0],
                    ps_s[1][0:32, :],
                    AF.Exp,
                    bias=bias0[0:32, :],
                    scale=SCALE,
                ).then_inc(s_a, 1)
                nc.scalar.activation(
                    e_hi[:, OFF1 : OFF1 + N1],
                    ps_s[2][:, :],
                    AF.Exp,
                    bias=bias0[:, :],
                    scale=SCALE,
                ).then_inc(s_a, 1)
                nc.scalar.activation(
                    e_lo[:, OFF1 : OFF1 + N1],
                    ps_s[3][0:32, :],
                    AF.Exp,
                    bias=bias0[0:32, :],
                    scale=SCALE,
                ).then_inc(s_a, 1)  # s_a = 4
                # copy transposed chunks into f_T = e - 1 (cast to bf16)
                for k, (t, m, _thr) in enumerate(tr_list):
                    m0, msz = (0, 128) if m == 0 else (128, 32)
                    act.wait_ge(s_p, 2 + k)
                    nc.scalar.activation(
                        eT_sb[:, t, m0 : m0 + msz],
                        ps_t[k % 4][0:128, 0:msz],
                        AF.Copy,
                        bias=-1.0,
                    ).then_inc(s_a, 1)  # s_a = 5+k
                # copy phase-2 accumulators to out staging
                for gidx, (m, n) in enumerate(groups):
                    msz = 128 if m == 0 else 32
                    dst = (
                        o_hi[:, n * 512 : (n + 1) * 512]
                        if m == 0
                        else o_lo[:, n * 512 : (n + 1) * 512]
                    )
                    act.wait_ge(s_p, 14 + gidx)
                    nc.scalar.activation(
                        dst, ps_t[gidx % 4][0:msz, 0:512], AF.Copy
                    ).then_inc(s_a, 1)  # s_a = 17+gidx

            @block.vector
            def _(dve):
                dve.wait_ge(s_a, 4)
                nc.vector.reduce_sum(
                    dh_sb[:], e_hi[:, :], axis=mybir.AxisListType.X
                ).then_inc(s_v, 1)
                nc.vector.reduce_sum(
                    dl_sb[:], e_lo[:, :], axis=mybir.AxisListType.X
                ).then_inc(s_v, 1)

    return nc


def _host_prep(z1_hat, z2):
    z1 = np.asarray(z1_hat, dtype=np.float32)[0]  # [128, 100, 64]
    z2a = np.asarray(z2, dtype=np.float32)[0]

    # q patches [160, 5120] and lhsT layout qT3 [128, 40, 160]
    q = z1.reshape(KC, NH, KH, NW, KW).transpose(1, 3, 0, 2, 4).reshape(PQ, D)
    qT3 = np.ascontiguousarray(
        q.reshape(PQ, KC, KH * KW).transpose(1, 2, 0).astype(P1_NP)
    )

    # padded z2: rows 100..111 zero
    z_pad = np.zeros((KC, 112, W), dtype=np.float32)
    z_pad[:, :H] = z2a

    # sliding kv patches from padded z2
    sw = np.lib.stride_tricks.sliding_window_view(z_pad, (KH, KW), axis=(1, 2))
    # sw: [128, 103, 61, 10, 4]; patch(h, w) = sw[:, h, w]

    q64 = q.astype(np.float64)
    ij_off = (np.arange(KH)[:, None] * W + np.arange(KW)[None, :]).reshape(-1)  # [40]

    in_maps = []
    corrs = []
    for core in range(NCORES):
        h0 = HPC * core
        zf = z_pad[:, h0 : h0 + ZROWS, :].reshape(KC, ZROWS * W)
        # 4 byte-shifted slabs for 128B-aligned phase-1 streams
        z4 = np.zeros((KC, KW, ZROWS * W), dtype=np.float32)
        for s in range(KW):
            z4[:, s, : ZROWS * W - s] = zf[:, s:]
        # kv rows indexed by flat position p = h_local*64 + w
        kvp = np.zeros((PKC, D), dtype=np.float32)
        hh = np.arange(PKC) // W
        ww = np.arange(PKC) % W
        real = (ww < WK) & (h0 + hh < HK)
        ridx = np.nonzero(real)[0]
        kvp[ridx] = (
            sw[:, h0 + hh[ridx], ww[ridx]].transpose(1, 0, 2, 3).reshape(-1, D)
        )
        kvr = np.ascontiguousarray(
            kvp.reshape(T, 128, D).transpose(1, 0, 2).astype(ml_dtypes.bfloat16)
        )
        in_maps.append(
            {
                "z": np.ascontiguousarray(z4.astype(P1_NP)),
                "qT3": qT3,
                "kvr": kvr,
            }
        )
        # denominator correction: computed-but-invalid columns. The device
        # computes exp(q . window / D) for every position in the two
        # contiguous streams [0,445) and [448,765); positions that are not
        # real patches (w >= 61 or h >= 91) polluted the on-chip row-sum.
        # streams now cover every flat position; invalid = not a real patch
        bad = np.nonzero(~real)[0]
        win = zf.astype(np.float64)[:, bad[:, None] + ij_off[None, :]]  # [128,nb,40]
        patches = win.transpose(1, 0, 2).reshape(len(bad), D)  # d-order (c, i, j)
        s_bad = q64 @ patches.T  # [160, nb]
        corrs.append(np.exp(s_bad * SCALE).sum(axis=1))

    corr = np.sum(corrs, axis=0)
    # centered softmax: device returns f @ kv with f = e - 1; host adds the
    # exact colsum term sum_k kv[k, :] over all real patches (all cores).
    swr = sw[:, :HK, :WK]
    colsum = swr.astype(np.float64).sum(axis=(1, 2)).reshape(D)  # [5120]
    return in_maps, corr, colsum


def kernel(z1_hat, z2):
    from concourse.bass_utils import run_bass_kernel_spmd

    in_maps, corr, colsum = _host_prep(z1_hat, z2)
    if "nc" not in _CACHE:
        _CACHE["nc"] = _build_nc()
    nc = _CACHE["nc"]
    res = run_bass_kernel_spmd(nc, in_maps, list(range(NCORES)))
    num = np.broadcast_to(colsum, (PQ, D)).astype(np.float64).copy()
    den = -corr
    for r in res.results:
        num += r["out"].astype(np.float64)
        den = den + r["den"].astype(np.float64)[:, 0]
    out = (num / den[:, None]).astype(np.float32)
    # fold patches back: [160, 5120] -> [1, 128, 100, 64]
    out = out.reshape(NH, NW, KC, KH, KW).transpose(2, 0, 3, 1, 4)
    return np.ascontiguousarray(out.reshape(1, KC, H, W))
